# revision 29
# baseline (speedup 1.0000x reference)
"""Trainium2 Bass kernel for nn_Attention_5463198400554.

Reference computation (per batch b of 8):
    q    = Wq @ x[b]                      # (N, C) contraction over x's first axis
    attn = scale * q @ x[b].T             # (N, N) contraction over x's second axis
    m    = rowmax(attn)                   # (N, 1)
    v    = colmean(x[b])                  # (1, C)  (mean over tokens)
    out[b][i][j] = v[i] * m[j]            # outer product, (C, N) == (N, C)

Strategy: pure data-parallel over batch — 8 batches on 8 NeuronCores, no
collectives. Key algebraic move: attn = scale * Wq @ G with G = x @ x.T
symmetric, so q is never computed; only G's upper block-triangle is built
by matmul (40 of 64 [128,512] chunks) and the strictly-lower 128-blocks
are mirrored by PE transposes. All transposes (x, Wq, G-lower) run as
regular bf16 matmuls against an identity (out = block.T @ I), which
pipelines at the N=128 matmul rate instead of the slow transpose mode.

Pipeline (single TileContext; Tile owns all semaphores):
  1. x streams in (f32), casts split across ACT/DVE, and is transposed
     tile-by-tile into xT — no resident natural copy of x.
  2. G upper chunks (psum accum over c-blocks) are emitted interleaved
     with the Wq stage (quarter-row staging + transposes into WqT) and
     with the G-lower mirrors as their dependencies complete, so the PE
     stream stays dense; chunk order follows x-group availability.
     v (column sums of x) reduces on ACT via activation accum_out.
  3. attn row-blocks: psum[n-block, m] = sum_i WqT[i, n-block].T @ G[i, m];
     rowmax straight out of PSUM on VectorE. Per-n-block column-wise
     epilogue (DVE 32x32 stream-transpose of m + DRAM broadcast bounce,
     one fused scalar_tensor_tensor, one 1MB output DMA) pipelines with
     the remaining attn matmuls.

The walrus build here caps sync waits at 1 per instruction (2 for
EventSemaphore); _legalize_wait_counts splits Tile's over-capacity waits
onto injected same-engine EventSemaphore carriers post-scheduling.
"""

from contextlib import ExitStack

import numpy as np

import concourse.bass as bass
import concourse.tile as tile
from concourse import mybir
from concourse.bass_utils import run_bass_kernel_spmd
from concourse.masks import make_identity


def _legalize_wait_counts(nc: bass.Bass) -> None:
    """Split over-capacity sync waits onto injected EventSemaphore carriers.

    This walrus build rejects instructions carrying more sync waits than the
    ISA struct holds ("Too many sync wait commands"): 1 wait for ordinary
    instructions, 2 for EventSemaphore. Tile's wait assignment emits more
    (e.g. WAR + RAW on one DMA, or the kernel-tail Drain waiting on every
    DMA queue). Moving excess waits to same-engine EventSemaphore carriers
    immediately before the instruction preserves ordering: the engine blocks
    until those semaphores reach their thresholds, then issues the original
    instruction with the remaining wait.
    """
    counter = [0]
    for blk in nc.m.functions[0].blocks:
        new_insts = []
        changed = False
        for ins in blk.instructions:
            si = ins.sync_info
            waits = list(si.on_wait) if si is not None else []
            cap = 2 if isinstance(ins, mybir.InstEventSemaphore) else 1
            if len(waits) > cap:
                changed = True
                excess, keep = waits[:-cap], waits[-cap:]
                for s in range(0, len(excess), 2):
                    counter[0] += 1
                    ev = mybir.InstEventSemaphore(
                        name=f"waitsplit-{counter[0]}", ins=[], outs=[]
                    )
                    ev.engine = ins.engine
                    ev.sync_info = mybir.SyncInfo(
                        on_wait=excess[s : s + 2], on_update=[]
                    )
                    new_insts.append(ev)
                ins.sync_info = mybir.SyncInfo(
                    on_wait=keep, on_update=list(si.on_update)
                )
            new_insts.append(ins)
        if changed:
            blk.instructions = new_insts

B = 8
N = 2048  # tokens == channels == dim
P = 128  # partitions
NB = N // P  # 16 blocks of 128
OC = 512  # matmul moving-operand chunk (one PSUM bank of f32)
NOC = N // OC  # 4 chunks
NUM_HEADS = 8
SCALE = (N // NUM_HEADS) ** -0.5  # 1/16
OUT_CONST = SCALE / N  # folds attn scale and the v-mean divisor

F32 = mybir.dt.float32
BF16 = mybir.dt.bfloat16


def build_graph(reps: int = 1) -> bass.Bass:
    nc = bass.Bass(trn_type="TRN2", target_bir_lowering=False, debug=False)
    x_ext = nc.dram_tensor("x", [N, N], F32, kind="ExternalInput").ap()
    wq_ext = nc.dram_tensor("wq", [N, N], F32, kind="ExternalInput").ap()
    out_ext = nc.dram_tensor("out", [N, N], F32, kind="ExternalOutput").ap()

    with tile.TileContext(nc) as tc, ExitStack() as octx:
        consts = octx.enter_context(tc.tile_pool(name="consts", bufs=1))
        ident_bf = consts.tile([P, P], BF16, name="ident_bf")
        make_identity(nc, ident_bf)
        ident_f32 = consts.tile([P, P], F32, name="ident_f32")
        make_identity(nc, ident_f32)
        for rep in range(reps):
            _emit_body(nc, tc, x_ext, wq_ext, out_ext, ident_bf, ident_f32, rep)

    _legalize_wait_counts(nc)
    return nc


def _emit_body(nc, tc, x_ext, wq_ext, out_ext, ident_bf, ident_f32, rep):
    """v3: attn = scale * Wq @ G with G = x @ x.T (symmetric).

    Emission order puts G's matmuls ahead of the Wq stage so the PE fills
    with G work while Wq streams in through small quarter-row staging tiles
    (the only SBUF left once xt + wqt + G are resident). G's strictly-lower
    128-blocks are mirrored from the upper chunks by PE transposes. The
    epilogue is column-wise and pipelines with the attn phase.
    """
    R = f"r{rep}_"
    with ExitStack() as ctx:
        stats = ctx.enter_context(tc.tile_pool(name=R + "stats", bufs=1))
        dram = ctx.enter_context(tc.tile_pool(name=R + "dram", bufs=16, space="DRAM"))

        v_all = stats.tile([P, NB], F32, name=R + "v_all")  # column sums of x
        m_all = stats.tile([P, NB], F32, name=R + "m_all")  # row maxes

        wqt_pool = ctx.enter_context(
            tc.tile_pool(name=R + "wqt", bufs=1, side="right")
        )
        wqt = wqt_pool.tile([P, NB, N], BF16, name=R + "wqt")  # WqT[i, n]
        g = None

        with tc.tile_pool(name=R + "xt", bufs=1) as xt_pool:
            xt = xt_pool.tile([P, NB, N], BF16, name=R + "xt")  # xT[c, m]

            # ---- load x, cast, transpose into xt ----
            # psX (transposes) and psG (G accumulation) coexist so G chunks
            # can start filling PE gaps while later x-groups still stream in.
            pctx = ExitStack()
            psx_pool = pctx.enter_context(
                tc.tile_pool(name=R + "psX", bufs=3, space="PSUM")
            )
            psg_pool = pctx.enter_context(
                tc.tile_pool(name=R + "psG", bufs=3, space="PSUM")
            )
            with (
                tc.tile_pool(name=R + "xs", bufs=4) as xs_pool,
                tc.tile_pool(name=R + "xb", bufs=6) as xb_pool,
            ):
                for ig in range(4):  # groups of 4 x row-blocks
                    xbs = []
                    for k in range(4):
                        i = ig * 4 + k
                        xs = xs_pool.tile([P, N], F32, tag="xs", name=f"{R}xs{i}")
                        nc.sync.dma_start(xs[:], x_ext[i * P : (i + 1) * P, :])
                        xb = xb_pool.tile([P, N], BF16, tag="xb", name=f"{R}xb{i}")
                        # split casts ACT/DVE so neither engine caps the stage
                        if i % 3 == 2:
                            nc.vector.tensor_copy(xb[:], xs[:])
                        else:
                            nc.scalar.copy(xb[:], xs[:])
                        xbs.append(xb)
                    for s in range(NB):
                        pt = psx_pool.tile(
                            [P, OC], F32, tag="pt", name=f"{R}pt{ig}_{s}"
                        )
                        for k in range(4):
                            nc.tensor.matmul(
                                pt[:, k * P : (k + 1) * P],
                                xbs[k][:, s * P : (s + 1) * P],
                                ident_bf[:],
                                start=True,
                                stop=True,
                            )
                        nc.vector.tensor_copy(
                            xt[:, s, ig * OC : (ig + 1) * OC], pt[:]
                        )

            # ---- G = x @ x.T upper chunks; Wq stage emitted after so the
            #      PE prefers G matmuls while Wq DMA streams ----
            g_pool = ctx.enter_context(
                tc.tile_pool(name=R + "g", bufs=1, side="right")
            )
            g = g_pool.tile([P, NB, N], BF16, name=R + "g")  # G[n, m]
            with (
                tc.tile_pool(name=R + "wqs", bufs=2) as wqs_pool,
                tc.tile_pool(name=R + "wqb", bufs=5) as wqb_pool,
            ):
                QW = OC  # 512-column quarters

                def emit_g_chunk(a, bc):
                    pg = psg_pool.tile([P, OC], F32, tag="pg", name=f"{R}pg{a}_{bc}")
                    for cb in range(NB):
                        nc.tensor.matmul(
                            pg[:],
                            xt[:, cb, a * P : (a + 1) * P],
                            xt[:, cb, bc * OC : (bc + 1) * OC],
                            start=(cb == 0),
                            stop=(cb == NB - 1),
                        )
                    nc.vector.tensor_copy(g[:, a, bc * OC : (bc + 1) * OC], pg[:])

                def emit_wq_unit(ig, q, unit):
                    wbq = []
                    for k in range(4):
                        o = ig * 4 + k
                        ws = wqs_pool.tile(
                            [P, QW], F32, tag="ws", name=f"{R}ws{o}_{q}"
                        )
                        nc.scalar.dma_start(
                            ws[:],
                            wq_ext[o * P : (o + 1) * P, q * QW : (q + 1) * QW],
                        )
                        wb = wqb_pool.tile(
                            [P, QW], BF16, tag="wb", name=f"{R}wb{o}_{q}"
                        )
                        # alternate cast engine so neither FIFO starves
                        if (unit * 4 + k) % 2 == 0:
                            nc.scalar.copy(wb[:], ws[:])
                        else:
                            nc.vector.tensor_copy(wb[:], ws[:])
                        wbq.append(wb)
                    for si in range(4):  # i-block s = 4q + si
                        s = 4 * q + si
                        pw = psx_pool.tile(
                            [P, OC], F32, tag="pt", name=f"{R}pw{ig}_{s}"
                        )
                        for k in range(4):
                            nc.tensor.matmul(
                                pw[:, k * P : (k + 1) * P],
                                wbq[k][:, si * P : (si + 1) * P],
                                ident_bf[:],
                                start=True,
                                stop=True,
                            )
                        nc.vector.tensor_copy(
                            wqt[:, s, ig * OC : (ig + 1) * OC], pw[:]
                        )

                def emit_g_low(a, bg):
                    pl = psg_pool.tile(
                        [P, OC], F32, tag="pg", name=f"{R}pl{a}_{bg}"
                    )
                    for k in range(4):
                        b = bg * 4 + k
                        nc.tensor.matmul(
                            pl[:, k * P : (k + 1) * P],
                            g[:, b, a * P : (a + 1) * P],
                            ident_bf[:],
                            start=True,
                            stop=True,
                        )
                    nc.vector.tensor_copy(g[:, a, bg * OC : (bg + 1) * OC], pl[:])

                # ordered so chunk (a, bc) is emitted once x-groups
                # max(a//4, bc) have landed -> G starts after group 0
                g_chunks = sorted(
                    ((a, bc) for a in range(NB) for bc in range(a // 4, NOC)),
                    key=lambda t: (max(t[0] // 4, t[1]), t[1], t[0]),
                )
                wq_units = [(ig, q) for ig in range(4) for q in range(4)]
                # lower-mirror group (a, bg) depends on upper chunks
                # (b, a//4) for b in 4bg..4bg+3
                low_pending = [
                    (a, bg) for a in range(NB) for bg in range((4 * (a // 4)) // 4)
                ]
                done_chunks = set()

                def flush_low():
                    nonlocal low_pending
                    rest = []
                    for a, bg in low_pending:
                        deps = {(4 * bg + k, a // 4) for k in range(4)}
                        if deps <= done_chunks:
                            emit_g_low(a, bg)
                        else:
                            rest.append((a, bg))
                    low_pending = rest

                gi = 0
                for u, (ig, q) in enumerate(wq_units):
                    for _ in range(3 if u % 2 == 0 else 2):  # 3+2 alternating = 40
                        if gi < len(g_chunks):
                            emit_g_chunk(*g_chunks[gi])
                            done_chunks.add(g_chunks[gi])
                            gi += 1
                    flush_low()
                    emit_wq_unit(ig, q, u)
                while gi < len(g_chunks):
                    emit_g_chunk(*g_chunks[gi])
                    done_chunks.add(g_chunks[gi])
                    gi += 1
                    flush_low()
                assert not low_pending

            # v: column sums of x == row sums of xT (consumed by epilogue).
            # Runs on ACT (idle once Wq is staged) via activation accum_out,
            # keeping DVE free for the G-lower evacs that gate attn start.
            with tc.tile_pool(name=R + "vscr", bufs=2) as vscr_pool:
                for s in range(NB):
                    vs = vscr_pool.tile([P, N], BF16, tag="vs", name=f"{R}vs{s}")
                    nc.scalar.activation(
                        out=vs[:],
                        in_=xt[:, s, :],
                        func=mybir.ActivationFunctionType.Copy,
                        accum_out=v_all[:, s : s + 1],
                    )

        pctx.close()

        # ---- attn rows = Wq @ G (scaled), rowmax, column-wise epilogue ----
        with (
            tc.tile_pool(name=R + "psB", bufs=2, space="PSUM") as psb_pool,
            tc.tile_pool(name=R + "epi", bufs=3) as epi_pool,
            tc.tile_pool(name=R + "ot", bufs=4) as ot_pool,
        ):
            for nb in range(NB):
                pb = psb_pool.tile([P, N], F32, tag="pb", name=f"{R}pb{nb}")
                for ib in range(NB):
                    for mc in range(NOC):
                        nc.tensor.matmul(
                            pb[:, mc * OC : (mc + 1) * OC],
                            wqt[:, ib, nb * P : (nb + 1) * P],
                            g[:, ib, mc * OC : (mc + 1) * OC],
                            start=(ib == 0),
                            stop=(ib == NB - 1),
                        )
                # m column -> row strips via DVE 32x32 stream transpose:
                # mt[32b, c] = m[32b + c]; 4-descriptor DMA to a DRAM row,
                # then partition-broadcast load back.
                mt_in = epi_pool.tile([P, 32], F32, tag="mti", name=f"{R}mti{nb}")
                nc.vector.reduce_max(
                    out=mt_in[:, 0:1], in_=pb[:], axis=mybir.AxisListType.X
                )
                mt = epi_pool.tile([P, 32], F32, tag="mt", name=f"{R}mt{nb}")
                nc.vector.transpose(mt[:], mt_in[:])
                md = dram.tile([1, P], F32, tag="md", name=f"{R}md{nb}")
                strips = bass.AP(
                    tensor=mt.tensor, offset=mt.offset, ap=[[32 * mt.ap[0][0], 4], [1, 32]]
                )
                nc.sync.dma_start(md[0, :].rearrange("(a b) -> a b", a=4), strips)
                m_bc = epi_pool.tile([P, P], F32, tag="mbc", name=f"{R}mb{nb}")
                nc.sync.dma_start(
                    m_bc[:],
                    bass.AP(tensor=md.tensor, offset=md.offset, ap=[[0, P], [1, P]]),
                )
                # out[:, nb-block] = v[i] * OUT_CONST * m[j], one fused DVE op
                # (m_bc broadcast over ib via step-0; v broadcast over j)
                ot = ot_pool.tile([P, NB, P], F32, tag="ot", name=f"{R}ot{nb}")
                m_in = bass.AP(
                    tensor=m_bc.tensor,
                    offset=m_bc.offset,
                    ap=[m_bc.ap[0], [0, NB], [1, P]],
                )
                v_in = bass.AP(
                    tensor=v_all.tensor,
                    offset=v_all.offset,
                    ap=[v_all.ap[0], [1, NB], [0, P]],
                )
                nc.vector.scalar_tensor_tensor(
                    out=ot[:],
                    in0=m_in,
                    scalar=OUT_CONST,
                    in1=v_in,
                    op0=mybir.AluOpType.mult,
                    op1=mybir.AluOpType.mult,
                )
                nc.sync.dma_start(
                    out_ext[:, nb * P : (nb + 1) * P].rearrange(
                        "(ib p) j -> p ib j", p=P
                    ),
                    ot[:],
                )


_NC_CACHE = None


def _get_graph() -> bass.Bass:
    global _NC_CACHE
    if _NC_CACHE is None:
        _NC_CACHE = build_graph()
    return _NC_CACHE


def kernel(x=None, Wq=None, H=None, W=None, **_ignored) -> np.ndarray:
    """Full-input entry point: x (8, 2048, 2048) f32, Wq (2048, 2048) f32.

    Shards batch elements across the 8 NeuronCores (data parallel), runs the
    Bass kernel SPMD, and stacks the per-core outputs back to (8, 2048, 2048).
    H and W are unused by the computation (the reference ignores them).
    """
    x = np.ascontiguousarray(np.asarray(x, dtype=np.float32))
    wq = np.ascontiguousarray(np.asarray(Wq, dtype=np.float32))
    assert x.shape == (B, N, N) and wq.shape == (N, N)

    nc = _get_graph()
    in_maps = [{"x": x[c], "wq": wq} for c in range(B)]
    res = run_bass_kernel_spmd(nc, in_maps, core_ids=list(range(B)))
    return np.stack([res.results[c]["out"] for c in range(B)], axis=0)


if __name__ == "__main__":
    rng = np.random.default_rng(0)
    x = rng.standard_normal((B, N, N), dtype=np.float32)
    wq = (rng.standard_normal((N, N), dtype=np.float32) * 0.02).astype(np.float32)
    out = kernel(x=x, Wq=wq, H=64, W=32)
    print("out shape:", out.shape, out.dtype)


# revision 30
# speedup vs baseline: 1.2779x; 1.2779x over previous
"""Trainium2 Bass kernel for nn_Attention_5463198400554.

Reference computation (per batch b of 8):
    q    = Wq @ x[b]                      # (N, C) contraction over x's first axis
    attn = scale * q @ x[b].T             # (N, N) contraction over x's second axis
    m    = rowmax(attn)                   # (N, 1)
    v    = colmean(x[b])                  # (1, C)  (mean over tokens)
    out[b][i][j] = v[i] * m[j]            # outer product, (C, N) == (N, C)

Strategy: pure data-parallel over batch — 8 batches on 8 NeuronCores, no
collectives. Key algebraic move: attn = scale * Wq @ G with G = x @ x.T
symmetric, so q is never computed; only G's upper block-triangle is built
by matmul (40 of 64 [128,512] chunks) and the strictly-lower 128-blocks
are mirrored by PE transposes. All transposes (x, Wq, G-lower) run as
regular bf16 matmuls against an identity (out = block.T @ I), which
pipelines at the N=128 matmul rate instead of the slow transpose mode.

Pipeline (single TileContext; Tile owns all semaphores):
  1. x streams in (f32), casts split across ACT/DVE, and is transposed
     tile-by-tile into xT — no resident natural copy of x.
  2. G upper chunks (psum accum over c-blocks) are emitted interleaved
     with the Wq stage (quarter-row staging + transposes into WqT) and
     with the G-lower mirrors as their dependencies complete, so the PE
     stream stays dense; chunk order follows x-group availability.
     v (column sums of x) reduces on ACT via activation accum_out.
  3. attn row-blocks: psum[n-block, m] = sum_i WqT[i, n-block].T @ G[i, m];
     rowmax straight out of PSUM on VectorE. Per-n-block column-wise
     epilogue (DVE 32x32 stream-transpose of m + DRAM broadcast bounce,
     one fused scalar_tensor_tensor, one 1MB output DMA) pipelines with
     the remaining attn matmuls.

The walrus build here caps sync waits at 1 per instruction (2 for
EventSemaphore); _legalize_wait_counts splits Tile's over-capacity waits
onto injected same-engine EventSemaphore carriers post-scheduling.
"""

from contextlib import ExitStack

import numpy as np

import concourse.bass as bass
import concourse.tile as tile
from concourse import mybir
from concourse.bass_utils import run_bass_kernel_spmd
from concourse.masks import make_identity


def _legalize_wait_counts(nc: bass.Bass) -> None:
    """Split over-capacity sync waits onto injected EventSemaphore carriers.

    This walrus build rejects instructions carrying more sync waits than the
    ISA struct holds ("Too many sync wait commands"): 1 wait for ordinary
    instructions, 2 for EventSemaphore. Tile's wait assignment emits more
    (e.g. WAR + RAW on one DMA, or the kernel-tail Drain waiting on every
    DMA queue). Moving excess waits to same-engine EventSemaphore carriers
    immediately before the instruction preserves ordering: the engine blocks
    until those semaphores reach their thresholds, then issues the original
    instruction with the remaining wait.
    """
    counter = [0]
    for blk in nc.m.functions[0].blocks:
        new_insts = []
        changed = False
        for ins in blk.instructions:
            si = ins.sync_info
            waits = list(si.on_wait) if si is not None else []
            cap = 2 if isinstance(ins, mybir.InstEventSemaphore) else 1
            if len(waits) > cap:
                changed = True
                excess, keep = waits[:-cap], waits[-cap:]
                for s in range(0, len(excess), 2):
                    counter[0] += 1
                    ev = mybir.InstEventSemaphore(
                        name=f"waitsplit-{counter[0]}", ins=[], outs=[]
                    )
                    ev.engine = ins.engine
                    ev.sync_info = mybir.SyncInfo(
                        on_wait=excess[s : s + 2], on_update=[]
                    )
                    new_insts.append(ev)
                ins.sync_info = mybir.SyncInfo(
                    on_wait=keep, on_update=list(si.on_update)
                )
            new_insts.append(ins)
        if changed:
            blk.instructions = new_insts

B = 8
N = 2048  # tokens == channels == dim
P = 128  # partitions
NB = N // P  # 16 blocks of 128
OC = 512  # matmul moving-operand chunk (one PSUM bank of f32)
NOC = N // OC  # 4 chunks
NUM_HEADS = 8
SCALE = (N // NUM_HEADS) ** -0.5  # 1/16
OUT_CONST = SCALE / N  # folds attn scale and the v-mean divisor

F32 = mybir.dt.float32
BF16 = mybir.dt.bfloat16


def build_graph(reps: int = 1) -> bass.Bass:
    nc = bass.Bass(trn_type="TRN2", target_bir_lowering=False, debug=False)
    x_ext = nc.dram_tensor("x", [N, N], F32, kind="ExternalInput").ap()
    wq_ext = nc.dram_tensor("wq", [N, N], F32, kind="ExternalInput").ap()
    out_ext = nc.dram_tensor("out", [N, N], F32, kind="ExternalOutput").ap()

    with tile.TileContext(nc) as tc, ExitStack() as octx:
        consts = octx.enter_context(tc.tile_pool(name="consts", bufs=1))
        ident_bf = consts.tile([P, P], BF16, name="ident_bf")
        make_identity(nc, ident_bf)
        ident_f32 = consts.tile([P, P], F32, name="ident_f32")
        make_identity(nc, ident_f32)
        for rep in range(reps):
            _emit_body(nc, tc, x_ext, wq_ext, out_ext, ident_bf, ident_f32, rep)

    _legalize_wait_counts(nc)
    return nc


def _emit_body(nc, tc, x_ext, wq_ext, out_ext, ident_bf, ident_f32, rep):
    """v3: attn = scale * Wq @ G with G = x @ x.T (symmetric).

    Emission order puts G's matmuls ahead of the Wq stage so the PE fills
    with G work while Wq streams in through small quarter-row staging tiles
    (the only SBUF left once xt + wqt + G are resident). G's strictly-lower
    128-blocks are mirrored from the upper chunks by PE transposes. The
    epilogue is column-wise and pipelines with the attn phase.
    """
    R = f"r{rep}_"
    with ExitStack() as ctx:
        stats = ctx.enter_context(tc.tile_pool(name=R + "stats", bufs=1))
        dram = ctx.enter_context(tc.tile_pool(name=R + "dram", bufs=16, space="DRAM"))

        v_all = stats.tile([P, NB], F32, name=R + "v_all")  # column sums of x
        m_all = stats.tile([P, NB], F32, name=R + "m_all")  # row maxes

        wqt_pool = ctx.enter_context(
            tc.tile_pool(name=R + "wqt", bufs=1, side="right")
        )
        wqt = wqt_pool.tile([P, NB, N], BF16, name=R + "wqt")  # WqT[i, n]
        g = None

        with tc.tile_pool(name=R + "xt", bufs=1) as xt_pool:
            xt = xt_pool.tile([P, NB, N], BF16, name=R + "xt")  # xT[c, m]

            # ---- load x, cast, transpose into xt ----
            # psX (transposes) and psG (G accumulation) coexist so G chunks
            # can start filling PE gaps while later x-groups still stream in.
            pctx = ExitStack()
            psx_pool = pctx.enter_context(
                tc.tile_pool(name=R + "psX", bufs=3, space="PSUM")
            )
            psg_pool = pctx.enter_context(
                tc.tile_pool(name=R + "psG", bufs=3, space="PSUM")
            )
            with (
                tc.tile_pool(name=R + "xs", bufs=4) as xs_pool,
                tc.tile_pool(name=R + "xb", bufs=6) as xb_pool,
            ):
                for ig in range(4):  # groups of 4 x row-blocks
                    xbs = []
                    for k in range(4):
                        i = ig * 4 + k
                        xs = xs_pool.tile([P, N], F32, tag="xs", name=f"{R}xs{i}")
                        nc.sync.dma_start(xs[:], x_ext[i * P : (i + 1) * P, :])
                        xb = xb_pool.tile([P, N], BF16, tag="xb", name=f"{R}xb{i}")
                        # split casts ACT/DVE so neither engine caps the stage
                        if i % 3 == 2:
                            nc.vector.tensor_copy(xb[:], xs[:])
                        else:
                            nc.scalar.copy(xb[:], xs[:])
                        xbs.append(xb)
                    for s in range(NB):
                        pt = psx_pool.tile(
                            [P, OC], F32, tag="pt", name=f"{R}pt{ig}_{s}"
                        )
                        for k in range(4):
                            nc.tensor.matmul(
                                pt[:, k * P : (k + 1) * P],
                                xbs[k][:, s * P : (s + 1) * P],
                                ident_bf[:],
                                start=True,
                                stop=True,
                            )
                        nc.vector.tensor_copy(
                            xt[:, s, ig * OC : (ig + 1) * OC], pt[:]
                        )

            # ---- G = x @ x.T upper chunks; Wq stage emitted after so the
            #      PE prefers G matmuls while Wq DMA streams ----
            g_pool = ctx.enter_context(
                tc.tile_pool(name=R + "g", bufs=1, side="right")
            )
            g = g_pool.tile([P, NB, N], BF16, name=R + "g")  # G[n, m]
            with (
                tc.tile_pool(name=R + "wqs", bufs=2) as wqs_pool,
                tc.tile_pool(name=R + "wqb", bufs=5) as wqb_pool,
            ):
                QW = OC  # 512-column quarters

                def emit_g_chunk(a, bc):
                    # diagonal chunk starts at the diagonal block; the skipped
                    # sub-diagonal blocks are mirrored from column a instead
                    off = (a % 4) * P if bc == a // 4 else 0
                    pg = psg_pool.tile([P, OC], F32, tag="pg", name=f"{R}pg{a}_{bc}")
                    for cb in range(NB):
                        nc.tensor.matmul(
                            pg[:, off:OC],
                            xt[:, cb, a * P : (a + 1) * P],
                            xt[:, cb, bc * OC + off : (bc + 1) * OC],
                            start=(cb == 0),
                            stop=(cb == NB - 1),
                        )
                    nc.vector.tensor_copy(
                        g[:, a, bc * OC + off : (bc + 1) * OC], pg[:, off:OC]
                    )

                def emit_wq_unit(ig, q, unit):
                    wbq = []
                    for k in range(4):
                        o = ig * 4 + k
                        ws = wqs_pool.tile(
                            [P, QW], F32, tag="ws", name=f"{R}ws{o}_{q}"
                        )
                        nc.scalar.dma_start(
                            ws[:],
                            wq_ext[o * P : (o + 1) * P, q * QW : (q + 1) * QW],
                        )
                        wb = wqb_pool.tile(
                            [P, QW], BF16, tag="wb", name=f"{R}wb{o}_{q}"
                        )
                        # alternate cast engine so neither FIFO starves
                        if (unit * 4 + k) % 2 == 0:
                            nc.scalar.copy(wb[:], ws[:])
                        else:
                            nc.vector.tensor_copy(wb[:], ws[:])
                        wbq.append(wb)
                    for si in range(4):  # i-block s = 4q + si
                        s = 4 * q + si
                        pw = psx_pool.tile(
                            [P, OC], F32, tag="pt", name=f"{R}pw{ig}_{s}"
                        )
                        for k in range(4):
                            nc.tensor.matmul(
                                pw[:, k * P : (k + 1) * P],
                                wbq[k][:, si * P : (si + 1) * P],
                                ident_bf[:],
                                start=True,
                                stop=True,
                            )
                        nc.vector.tensor_copy(
                            wqt[:, s, ig * OC : (ig + 1) * OC], pw[:]
                        )

                def emit_g_low(a, bg, w):
                    pl = psg_pool.tile(
                        [P, OC], F32, tag="pg", name=f"{R}pl{a}_{bg}"
                    )
                    for k in range(w):
                        b = bg * 4 + k
                        nc.tensor.matmul(
                            pl[:, k * P : (k + 1) * P],
                            g[:, b, a * P : (a + 1) * P],
                            ident_bf[:],
                            start=True,
                            stop=True,
                        )
                    nc.vector.tensor_copy(
                        g[:, a, bg * OC : bg * OC + w * P], pl[:, 0 : w * P]
                    )

                # ordered so chunk (a, bc) is emitted once x-groups
                # max(a//4, bc) have landed -> G starts after group 0
                g_chunks = sorted(
                    ((a, bc) for a in range(NB) for bc in range(a // 4, NOC)),
                    key=lambda t: (max(t[0] // 4, t[1]), t[1], t[0]),
                )
                wq_units = [(ig, q) for ig in range(4) for q in range(4)]
                # lower-mirror group (a, bg, w) covers blocks b in
                # [4bg, 4bg+w); depends on upper chunks (b, a//4)
                low_pending = [
                    (a, bg, 4) for a in range(NB) for bg in range(a // 4)
                ]
                low_pending += [
                    (a, a // 4, a % 4) for a in range(NB) if a % 4 > 0
                ]
                done_chunks = set()

                def flush_low():
                    nonlocal low_pending
                    rest = []
                    for a, bg, w in low_pending:
                        deps = {(4 * bg + k, a // 4) for k in range(w)}
                        if deps <= done_chunks:
                            emit_g_low(a, bg, w)
                        else:
                            rest.append((a, bg, w))
                    low_pending = rest

                gi = 0
                for u, (ig, q) in enumerate(wq_units):
                    for _ in range(3 if u % 2 == 0 else 2):  # 3+2 alternating = 40
                        if gi < len(g_chunks):
                            emit_g_chunk(*g_chunks[gi])
                            done_chunks.add(g_chunks[gi])
                            gi += 1
                    flush_low()
                    emit_wq_unit(ig, q, u)
                while gi < len(g_chunks):
                    emit_g_chunk(*g_chunks[gi])
                    done_chunks.add(g_chunks[gi])
                    gi += 1
                    flush_low()
                assert not low_pending

            # v: column sums of x == row sums of xT (consumed by epilogue).
            # Runs on ACT (idle once Wq is staged) via activation accum_out,
            # keeping DVE free for the G-lower evacs that gate attn start.
            with tc.tile_pool(name=R + "vscr", bufs=2) as vscr_pool:
                for s in range(NB):
                    vs = vscr_pool.tile([P, N], BF16, tag="vs", name=f"{R}vs{s}")
                    nc.scalar.activation(
                        out=vs[:],
                        in_=xt[:, s, :],
                        func=mybir.ActivationFunctionType.Copy,
                        accum_out=v_all[:, s : s + 1],
                    )

        pctx.close()

        # ---- attn rows = Wq @ G (scaled), rowmax, column-wise epilogue ----
        with (
            tc.tile_pool(name=R + "psB", bufs=2, space="PSUM") as psb_pool,
            tc.tile_pool(name=R + "epi", bufs=3) as epi_pool,
            tc.tile_pool(name=R + "ot", bufs=4) as ot_pool,
        ):
            for nb in range(NB):
                pb = psb_pool.tile([P, N], F32, tag="pb", name=f"{R}pb{nb}")
                for ib in range(NB):
                    for mc in range(NOC):
                        nc.tensor.matmul(
                            pb[:, mc * OC : (mc + 1) * OC],
                            wqt[:, ib, nb * P : (nb + 1) * P],
                            g[:, ib, mc * OC : (mc + 1) * OC],
                            start=(ib == 0),
                            stop=(ib == NB - 1),
                        )
                # m column -> row strips via DVE 32x32 stream transpose:
                # mt[32b, c] = m[32b + c]; 4-descriptor DMA to a DRAM row,
                # then partition-broadcast load back.
                mt_in = epi_pool.tile([P, 32], F32, tag="mti", name=f"{R}mti{nb}")
                nc.vector.reduce_max(
                    out=mt_in[:, 0:1], in_=pb[:], axis=mybir.AxisListType.X
                )
                mt = epi_pool.tile([P, 32], F32, tag="mt", name=f"{R}mt{nb}")
                nc.vector.transpose(mt[:], mt_in[:])
                md = dram.tile([1, P], F32, tag="md", name=f"{R}md{nb}")
                strips = bass.AP(
                    tensor=mt.tensor, offset=mt.offset, ap=[[32 * mt.ap[0][0], 4], [1, 32]]
                )
                nc.sync.dma_start(md[0, :].rearrange("(a b) -> a b", a=4), strips)
                m_bc = epi_pool.tile([P, P], F32, tag="mbc", name=f"{R}mb{nb}")
                nc.sync.dma_start(
                    m_bc[:],
                    bass.AP(tensor=md.tensor, offset=md.offset, ap=[[0, P], [1, P]]),
                )
                # out[:, nb-block] = v[i] * OUT_CONST * m[j], one fused DVE op
                # (m_bc broadcast over ib via step-0; v broadcast over j)
                ot = ot_pool.tile([P, NB, P], F32, tag="ot", name=f"{R}ot{nb}")
                m_in = bass.AP(
                    tensor=m_bc.tensor,
                    offset=m_bc.offset,
                    ap=[m_bc.ap[0], [0, NB], [1, P]],
                )
                v_in = bass.AP(
                    tensor=v_all.tensor,
                    offset=v_all.offset,
                    ap=[v_all.ap[0], [1, NB], [0, P]],
                )
                nc.vector.scalar_tensor_tensor(
                    out=ot[:],
                    in0=m_in,
                    scalar=OUT_CONST,
                    in1=v_in,
                    op0=mybir.AluOpType.mult,
                    op1=mybir.AluOpType.mult,
                )
                nc.sync.dma_start(
                    out_ext[:, nb * P : (nb + 1) * P].rearrange(
                        "(ib p) j -> p ib j", p=P
                    ),
                    ot[:],
                )


_NC_CACHE = None


def _get_graph() -> bass.Bass:
    global _NC_CACHE
    if _NC_CACHE is None:
        _NC_CACHE = build_graph()
    return _NC_CACHE


def kernel(x=None, Wq=None, H=None, W=None, **_ignored) -> np.ndarray:
    """Full-input entry point: x (8, 2048, 2048) f32, Wq (2048, 2048) f32.

    Shards batch elements across the 8 NeuronCores (data parallel), runs the
    Bass kernel SPMD, and stacks the per-core outputs back to (8, 2048, 2048).
    H and W are unused by the computation (the reference ignores them).
    """
    x = np.ascontiguousarray(np.asarray(x, dtype=np.float32))
    wq = np.ascontiguousarray(np.asarray(Wq, dtype=np.float32))
    assert x.shape == (B, N, N) and wq.shape == (N, N)

    nc = _get_graph()
    in_maps = [{"x": x[c], "wq": wq} for c in range(B)]
    res = run_bass_kernel_spmd(nc, in_maps, core_ids=list(range(B)))
    return np.stack([res.results[c]["out"] for c in range(B)], axis=0)


if __name__ == "__main__":
    rng = np.random.default_rng(0)
    x = rng.standard_normal((B, N, N), dtype=np.float32)
    wq = (rng.standard_normal((N, N), dtype=np.float32) * 0.02).astype(np.float32)
    out = kernel(x=x, Wq=wq, H=64, W=32)
    print("out shape:", out.shape, out.dtype)
